# revision 1
# baseline (speedup 1.0000x reference)
"""Trainium2 Bass kernel for nn_MemResProjections (memory-residual attention).

Reference computation (B=4, S=2048, K=64, H=1024, fp32):
    normed = rmsnorm(hidden) * norm_w
    v_pool = concat([normed, memory], axis=1)            # (B, S+K, H)
    q = normed @ q_w.T ; k = v_pool @ k_w.T
    logits = q @ k.T / sqrt(H)  with causal mask on the local S block,
    memory columns fully visible
    attn = softmax(logits); h~ = attn @ v_pool
    alpha = sigmoid(hidden @ gate_w.T + gate_b)
    out = (1-alpha)*hidden + alpha*h~

Sharding: 8 cores = (batch b, half h) pairs; each core owns 1024 query rows.
Cores with h=1 see the first 1024 rows as a fully-visible "prefix"; cores with
h=0 get a zero prefix that is masked out via a per-core additive bias fused
into the exp() activation, keeping the SPMD program uniform.

Per-core dataflow (all matmuls in float32r = full-rate fp32, ~13-bit mantissa):
  A1: rmsnorm stats in natural layout; normed stripes PE-transposed into
      normedT (h on partitions); v = normed*norm_w spilled to DRAM scratch.
  A2: gate logits from normedT with the rstd factored out (gate uses raw x =
      normed * std, so scale the psum by std on eviction); sigmoid -> alpha
      spilled to DRAM.
  A3: qT = (q_w*norm_w).T-stationary @ normedT ; kT over own+prefix+memory.
  B:  scores^T tiles (t on partitions, 256 query cols) -> fused scale+mask+exp
      -> h~ accumulation (lhsT=exp^T) + denominator via ones-matmul; the
      [1,256] denominator hops to [128,2] natural layout with one SBUF DMA;
      output = x + alpha*(h~/den - x).
"""
import numpy as np

import concourse.bass as bass
import concourse.mybir as mybir
import concourse.tile as tile
from concourse.bass_utils import run_bass_kernel_spmd

F32 = mybir.dt.float32
F32R = mybir.dt.float32r
AFT = mybir.ActivationFunctionType

P = 128
H = 1024
S_OWN = 1024          # query rows per core
T_PREF = 1024         # prefix kv rows
T_MEM = 64            # memory kv rows
NJ = H // P           # h blocks
NS = S_OWN // P       # own stripes
SCALE = 1.0 / 32.0    # 1/sqrt(H)
EPS = 1e-6
NEG = -1.0e30

N_CORES = 8
B_FULL, S_FULL = 4, 2048


# ---------------------------------------------------------------- walrus fix
ENGINE_ATTR = {
    mybir.EngineType.PE: "tensor",
    mybir.EngineType.Activation: "scalar",
    mybir.EngineType.DVE: "vector",
    mybir.EngineType.Pool: "gpsimd",
    mybir.EngineType.SP: "sync",
}
DMA_OPS = ("InstDMACopy", "InstDMATranspose", "InstTensorLoad", "InstTensorSave",
           "InstCollectiveCompute")


def split_multi_waits(nc, limit=1, dma_limit=None):
    """This walrus build rejects engine instructions carrying more than one
    sem wait; hoist extras onto same-engine NOPs inserted just before."""
    n_split = 0
    for f in nc.m.functions:
        for blk in f.blocks:
            il = blk.instructions
            i = 0
            while i < len(il):
                ins = il[i]
                is_dma = type(ins).__name__ in DMA_OPS
                lim = dma_limit if is_dma else limit
                si = ins.sync_info
                waits = list(si.on_wait) if si is not None and si.on_wait else []
                if lim is not None and len(waits) > lim:
                    keep, extra = waits[:lim], waits[lim:]
                    si.on_wait.clear()
                    for w in keep:
                        si.on_wait.append(w)
                    eng = getattr(nc, ENGINE_ATTR[ins.engine])
                    for w in extra:
                        nop = eng.nop(nofuse=True, hint="wait_split")
                        nop.wait_op(bass.SemaphoreHandle(w.ant_name, w.id),
                                    w.wait_value, "sem-ge")
                        popped = nc.cur_bb.bb.instructions.pop()
                        assert popped.name == nop.ins.name
                        il.insert(i, nop.ins)
                        i += 1
                        n_split += 1
                i += 1
    return n_split


# ---------------------------------------------------------------- program
def build_nc():
    nc = bass.Bass()
    dp = lambda n, shp: nc.declare_dram_parameter(n, shp, F32, isOutput=False)
    x_own = dp("x_own", [S_OWN, H])
    x_pref = dp("x_pref", [T_PREF, H])
    mem = dp("mem", [T_MEM, H])
    memT = dp("memT", [H, T_MEM])
    qwT = dp("qwT", [H, H])        # (q_w * norm_w).T
    kwT = dp("kwT", [H, H])        # (k_w * norm_w).T
    gwT = dp("gwT", [H, H])        # gate_w.T
    w_bc = dp("w_bc", [P, H])      # norm_w broadcast
    b_bc = dp("b_bc", [P, H])      # gate_b broadcast
    pbias = dp("pbias", [P, 1])    # 0 (h=1) or -1e30 (h=0)
    onesc = dp("onesc", [P, 1])    # ones column (fp32r matmul operand)
    tri = dp("tri", [P, P])        # additive mask: 0 if col>=row else -1e30
    ident = dp("ident", [P, P])    # identity for PE transpose
    out = nc.declare_dram_parameter("out", [S_OWN, H], F32, isOutput=True)
    import os as _os
    DEBUG = _os.environ.get("DEBUG_KERNEL") == "1"
    if DEBUG:
        dbg_rden = nc.declare_dram_parameter("dbg_rden", [P, NS], F32,
                                             isOutput=True)

    v_dram = nc.dram_tensor("v_scratch", [S_OWN + T_PREF, H], F32)
    a_dram = nc.dram_tensor("alpha_scratch", [S_OWN, H], F32)

    T_ALL = S_OWN + T_PREF  # spill rows

    with tile.TileContext(nc) as tc:
        from contextlib import ExitStack
        with ExitStack() as ctx:
            # ---- long-lived pools
            const = ctx.enter_context(tc.tile_pool(name="const", bufs=1))
            proj = ctx.enter_context(tc.tile_pool(name="proj", bufs=1))

            eps_t = const.tile([P, 1], F32)
            nc.vector.memset(eps_t[:], EPS)
            ones_col = const.tile([P, 1], F32)
            nc.sync.dma_start(out=ones_col[:], in_=onesc[:])
            neg_t = const.tile([P, 1], F32)
            nc.vector.memset(neg_t[:], NEG)
            pb_t = const.tile([P, 1], F32)
            nc.sync.dma_start(out=pb_t[:], in_=pbias[:])
            tri_t = const.tile([P, P], F32)
            nc.sync.dma_start(out=tri_t[:], in_=tri[:])
            id_t = const.tile([P, P], F32R)
            nc.sync.dma_start(out=id_t[:], in_=ident[:].bitcast(F32R))
            std_all = const.tile([P, 16], F32)
            rstd_all = const.tile([P, 16], F32)
            rden = const.tile([P, NS], F32)
            memT_t = [const.tile([P, T_MEM], F32R, tag=f"memT{j}", name=f"memT{j}") for j in range(NJ)]
            for j in range(NJ):
                nc.sync.dma_start(out=memT_t[j][:],
                                  in_=memT[j * P:(j + 1) * P, :].bitcast(F32R))

            qT = [proj.tile([P, S_OWN], F32R, tag=f"qT{m}", name=f"qT{m}") for m in range(NJ)]
            kT = [proj.tile([P, S_OWN + T_PREF + T_MEM], F32R, tag=f"kT{m}", name=f"kT{m}")
                  for m in range(NJ)]

            # ================= phase A: norm, transpose, gate, projections
            with tc.tile_pool(name="aphase", bufs=1) as ap:
                normedT = [ap.tile([P, 2048], F32R, tag=f"nT{j}", name=f"nT{j}") for j in range(NJ)]

                # ---- A1: stats + normed + PE transpose + v spill
                with tc.tile_pool(name="a1s", bufs=2) as ast, \
                     tc.tile_pool(name="a1ps", bufs=4, space="PSUM") as aps:
                    w_bc_t = ast.tile([P, H], F32, bufs=1)
                    nc.sync.dma_start(out=w_bc_t[:], in_=w_bc[:])
                    sq = ast.tile([P, H], F32, bufs=1)  # shared Square scratch
                    for part in range(2):          # 0 = own, 1 = prefix
                        src = x_own if part == 0 else x_pref
                        for i in range(NS):
                            idx = part * NS + i
                            xt = ast.tile([P, H], F32, tag="xt")
                            nc.sync.dma_start(out=xt[:],
                                              in_=src[i * P:(i + 1) * P, :])
                            ss = ast.tile([P, 1], F32, tag="ss")
                            nc.scalar.activation(sq[:], xt[:], AFT.Square,
                                                 accum_out=ss[:])
                            nc.scalar.activation(std_all[:, idx:idx + 1], ss[:],
                                                 AFT.Sqrt, scale=1.0 / H,
                                                 bias=eps_t[:])
                            nc.vector.reciprocal(rstd_all[:, idx:idx + 1],
                                                 std_all[:, idx:idx + 1])
                            nrm = ast.tile([P, H], F32R, tag="nrm")
                            nc.scalar.activation(nrm[:], xt[:], AFT.Copy,
                                                 scale=rstd_all[:, idx:idx + 1])
                            # transpose 128x128 subtiles into normedT columns
                            for j in range(NJ):
                                tp = aps.tile([P, P], F32R, tag="tp")
                                nc.tensor.transpose(
                                    tp[:], nrm[:, j * P:(j + 1) * P], id_t[:])
                                nc.vector.tensor_copy(
                                    normedT[j][:, idx * P:(idx + 1) * P], tp[:])
                            # v = normed * norm_w (in place) -> spill
                            nc.vector.tensor_mul(nrm[:], nrm[:].bitcast(F32),
                                                 w_bc_t[:])
                            nc.sync.dma_start(
                                out=v_dram[idx * P:(idx + 1) * P, :],
                                in_=nrm[:].bitcast(F32))

                # ---- A2: gate -> alpha spill (gate = (normed @ gwT) * std)
                with tc.tile_pool(name="a2s", bufs=2) as gst, \
                     tc.tile_pool(name="a2ps", bufs=1, space="PSUM") as gps:
                    b_bc_t = gst.tile([P, H], F32, bufs=1)
                    nc.sync.dma_start(out=b_bc_t[:], in_=b_bc[:])
                    for oc in range(2):
                        pg = [gps.tile([P, 512], F32, tag=f"pg{si}",
                                       name=f"pg{si}") for si in range(NS)]
                        for j in range(NJ):
                            gwj = gst.tile([P, 512], F32R, tag="gwj")
                            nc.sync.dma_start(
                                out=gwj[:],
                                in_=gwT[j * P:(j + 1) * P,
                                        oc * 512:(oc + 1) * 512].bitcast(F32R))
                            for si in range(NS):
                                nc.tensor.matmul(
                                    pg[si][:],
                                    normedT[j][:, si * P:(si + 1) * P],
                                    gwj[:],
                                    start=(j == 0), stop=(j == NJ - 1))
                        for si in range(NS):
                            gl = gst.tile([P, 512], F32, tag="gl")
                            nc.scalar.activation(gl[:], pg[si][:], AFT.Copy,
                                                 scale=std_all[:, si:si + 1])
                            nc.vector.tensor_add(
                                gl[:], gl[:], b_bc_t[:, oc * 512:(oc + 1) * 512])
                            al = gst.tile([P, 512], F32, tag="al")
                            nc.scalar.activation(al[:], gl[:], AFT.Sigmoid)
                            nc.sync.dma_start(
                                out=a_dram[si * P:(si + 1) * P,
                                           oc * 512:(oc + 1) * 512],
                                in_=al[:])

                # ---- A3: qT / kT projections
                with tc.tile_pool(name="wstrip", bufs=2) as wsp, \
                     tc.tile_pool(name="a3ps", bufs=2, space="PSUM") as aps3:
                    for m in range(NJ):
                        qs = wsp.tile([P, H], F32R, tag="qs")
                        ks = wsp.tile([P, H], F32R, tag="ks")
                        for j in range(NJ):
                            nc.sync.dma_start(
                                out=qs[:, j * P:(j + 1) * P],
                                in_=qwT[j * P:(j + 1) * P,
                                        m * P:(m + 1) * P].bitcast(F32R))
                            nc.sync.dma_start(
                                out=ks[:, j * P:(j + 1) * P],
                                in_=kwT[j * P:(j + 1) * P,
                                        m * P:(m + 1) * P].bitcast(F32R))
                        # qT_m over own cols
                        for sc in range(2):
                            pq = aps3.tile([P, 512], F32, tag="pq")
                            for j in range(NJ):
                                nc.tensor.matmul(
                                    pq[:], qs[:, j * P:(j + 1) * P],
                                    normedT[j][:, sc * 512:(sc + 1) * 512],
                                    start=(j == 0), stop=(j == NJ - 1))
                            nc.vector.tensor_copy(
                                qT[m][:, sc * 512:(sc + 1) * 512], pq[:])
                        # kT_m over own+prefix cols
                        for sc in range(4):
                            pk = aps3.tile([P, 512], F32, tag="pq")
                            for j in range(NJ):
                                nc.tensor.matmul(
                                    pk[:], ks[:, j * P:(j + 1) * P],
                                    normedT[j][:, sc * 512:(sc + 1) * 512],
                                    start=(j == 0), stop=(j == NJ - 1))
                            nc.vector.tensor_copy(
                                kT[m][:, sc * 512:(sc + 1) * 512], pk[:])
                        # kT_m over memory cols
                        pkm = aps3.tile([P, T_MEM], F32, tag="pkm")
                        for j in range(NJ):
                            nc.tensor.matmul(pkm[:], ks[:, j * P:(j + 1) * P],
                                             memT_t[j][:],
                                             start=(j == 0), stop=(j == NJ - 1))
                        nc.vector.tensor_copy(kT[m][:, 2048:2048 + T_MEM], pkm[:])

            # ================= phase B: attention
            with tc.tile_pool(name="bres", bufs=1) as bres, \
                 tc.tile_pool(name="bstream", bufs=2) as bst, \
                 tc.tile_pool(name="bexp", bufs=3) as bexp, \
                 tc.tile_pool(name="bps", bufs=2, space="PSUM") as bps, \
                 tc.tile_pool(name="bph", bufs=1, space="PSUM") as bph:
                vpref = [bres.tile([P, H], F32R, tag=f"vp{t}", name=f"vp{t}") for t in range(8)]
                for t in range(8):
                    nc.sync.dma_start(
                        out=vpref[t][:],
                        in_=v_dram[S_OWN + t * P:S_OWN + (t + 1) * P, :]
                            .bitcast(F32R))
                vmem = bres.tile([T_MEM, H], F32R)
                nc.sync.dma_start(out=vmem[:], in_=mem[:].bitcast(F32R))

                NHG = 4  # half-groups of 2 stripes (256 query cols)
                for hg in range(NHG):
                    s0 = hg * 256
                    # tau blocks: (kind, index): own 0..2hg+1, prefix 0..7, mem
                    taus = ([("own", t) for t in range(2 * hg + 2)]
                            + [("pref", t) for t in range(8)]
                            + [("mem", 0)])
                    ph = {(sl, hc): bph.tile([P, 512], F32, tag=f"ph{sl}{hc}", name=f"ph{sl}{hc}")
                          for sl in range(2) for hc in range(2)}
                    pd = [bph.tile([P, 1], F32, tag=f"pd{sl}", name=f"pd{sl}")
                          for sl in range(2)]
                    for ti, (kind, t) in enumerate(taus):
                        first, last = ti == 0, ti == len(taus) - 1
                        rows = T_MEM if kind == "mem" else P
                        # scores^T [rows, 256]
                        ps = bps.tile([P, 256], F32, tag="ps")
                        if kind == "own":
                            koff = t * P
                        elif kind == "pref":
                            koff = S_OWN + t * P
                        else:
                            koff = 2048
                        for m in range(NJ):
                            nc.tensor.matmul(
                                ps[:rows, :], kT[m][:, koff:koff + rows],
                                qT[m][:, s0:s0 + 256],
                                start=(m == 0), stop=(m == NJ - 1))
                        # exp with fused scale (+mask / prefix bias)
                        et = bexp.tile([P, 256], F32R, tag="et")
                        if kind == "own":
                            sl_d = t - 2 * hg
                            if sl_d == 0:
                                nc.vector.tensor_add(ps[:, 0:P], ps[:, 0:P],
                                                     tri_t[:])
                                nc.scalar.activation(et[:], ps[:], AFT.Exp,
                                                     scale=SCALE)
                            elif sl_d == 1:
                                nc.scalar.activation(et[:, 0:P], ps[:, 0:P],
                                                     AFT.Exp, scale=SCALE,
                                                     bias=neg_t[:])
                                nc.vector.tensor_add(ps[:, P:256], ps[:, P:256],
                                                     tri_t[:])
                                nc.scalar.activation(et[:, P:256], ps[:, P:256],
                                                     AFT.Exp, scale=SCALE)
                            else:
                                nc.scalar.activation(et[:], ps[:], AFT.Exp,
                                                     scale=SCALE)
                        elif kind == "pref":
                            nc.scalar.activation(et[:], ps[:], AFT.Exp,
                                                 scale=SCALE, bias=pb_t[:])
                        else:
                            nc.scalar.activation(et[:rows, :], ps[:rows, :],
                                                 AFT.Exp, scale=SCALE)
                        # v tile
                        if kind == "own":
                            vt = bst.tile([P, H], F32R, tag="vb", bufs=3)
                            nc.sync.dma_start(
                                out=vt[:],
                                in_=v_dram[t * P:(t + 1) * P, :].bitcast(F32R))
                        elif kind == "pref":
                            vt = vpref[t]
                        else:
                            vt = vmem
                        # h~ accumulation + denominator (same stationary et)
                        for sl in range(2):
                            for hc in range(2):
                                nc.tensor.matmul(
                                    ph[(sl, hc)][:],
                                    et[:rows, sl * P:(sl + 1) * P],
                                    vt[:rows, hc * 512:(hc + 1) * 512],
                                    start=first, stop=last,
                                    skip_group_check=True)
                            nc.tensor.matmul(
                                pd[sl][:],
                                et[:rows, sl * P:(sl + 1) * P].bitcast(F32),
                                ones_col[:rows, :],
                                start=first, stop=last,
                                skip_group_check=True)
                    for sl in range(2):
                        sidx = 2 * hg + sl
                        nc.vector.reciprocal(rden[:, sidx:sidx + 1], pd[sl][:])
                    # evict h~, final combine
                    for sl in range(2):
                        sidx = 2 * hg + sl
                        hsb = bst.tile([P, H], F32, tag="hsb")
                        for hc in range(2):
                            nc.scalar.activation(
                                hsb[:, hc * 512:(hc + 1) * 512], ph[(sl, hc)][:],
                                AFT.Copy, scale=rden[:, sidx:sidx + 1])
                        xs = bst.tile([P, H], F32, tag="xs")
                        nc.sync.dma_start(out=xs[:],
                                          in_=x_own[sidx * P:(sidx + 1) * P, :])
                        als = bst.tile([P, H], F32, tag="als")
                        nc.sync.dma_start(out=als[:],
                                          in_=a_dram[sidx * P:(sidx + 1) * P, :])
                        nc.vector.tensor_sub(hsb[:], hsb[:], xs[:])
                        nc.vector.tensor_mul(hsb[:], hsb[:], als[:])
                        nc.vector.tensor_add(hsb[:], hsb[:], xs[:])
                        nc.sync.dma_start(out=out[sidx * P:(sidx + 1) * P, :],
                                          in_=hsb[:])
                if DEBUG:
                    nc.sync.dma_start(out=dbg_rden[:], in_=rden[:])

    import os
    if os.environ.get("NO_WAIT_SPLIT") != "1":
        split_multi_waits(nc, limit=1, dma_limit=1)
    return nc


_NC_CACHE = None
_LAST_IN_MAPS = None


def _get_nc():
    global _NC_CACHE
    if _NC_CACHE is None:
        _NC_CACHE = build_nc()
    return _NC_CACHE


def prepare_in_maps(hidden_states, memory_state, q_w, k_w, norm_w, gate_w,
                    gate_b):
    hidden_states = np.asarray(hidden_states, dtype=np.float32)
    memory_state = np.asarray(memory_state, dtype=np.float32)
    q_w = np.asarray(q_w, dtype=np.float32)
    k_w = np.asarray(k_w, dtype=np.float32)
    norm_w = np.asarray(norm_w, dtype=np.float32)
    gate_w = np.asarray(gate_w, dtype=np.float32)
    gate_b = np.asarray(gate_b, dtype=np.float32)

    qwT = np.ascontiguousarray((q_w * norm_w[None, :]).T)
    kwT = np.ascontiguousarray((k_w * norm_w[None, :]).T)
    gwT = np.ascontiguousarray(gate_w.T)
    w_bc = np.ascontiguousarray(np.broadcast_to(norm_w, (P, H)))
    b_bc = np.ascontiguousarray(np.broadcast_to(gate_b, (P, H)))
    tri = np.where(np.arange(P)[None, :] >= np.arange(P)[:, None],
                   np.float32(0.0), np.float32(NEG)).astype(np.float32)
    ident = np.eye(P, dtype=np.float32)
    zeros_pref = np.zeros((T_PREF, H), dtype=np.float32)

    in_maps = []
    for c in range(N_CORES):
        b, h = divmod(c, 2)
        x_own = np.ascontiguousarray(hidden_states[b, h * S_OWN:(h + 1) * S_OWN])
        x_pref = (np.ascontiguousarray(hidden_states[b, :T_PREF]) if h == 1
                  else zeros_pref)
        memb = np.ascontiguousarray(memory_state[b])
        in_maps.append({
            "x_own": x_own,
            "x_pref": x_pref,
            "mem": memb,
            "memT": np.ascontiguousarray(memb.T),
            "qwT": qwT, "kwT": kwT, "gwT": gwT,
            "w_bc": w_bc, "b_bc": b_bc,
            "pbias": np.full((P, 1), 0.0 if h == 1 else NEG, np.float32),
            "onesc": np.ones((P, 1), np.float32),
            "tri": tri, "ident": ident,
        })
    return in_maps


def kernel(**inputs):
    in_maps = prepare_in_maps(**inputs)
    global _LAST_IN_MAPS
    _LAST_IN_MAPS = in_maps
    nc = _get_nc()
    res = run_bass_kernel_spmd(nc, in_maps, list(range(N_CORES)))
    out = np.empty((B_FULL, S_FULL, H), dtype=np.float32)
    for c in range(N_CORES):
        b, h = divmod(c, 2)
        out[b, h * S_OWN:(h + 1) * S_OWN] = res.results[c]["out"]
    return out



# revision 9
# speedup vs baseline: 2.0343x; 2.0343x over previous
"""Trainium2 Bass kernel for nn_MemResProjections (memory-residual attention).

Reference computation (B=4, S=2048, K=64, H=1024, fp32):
    normed = rmsnorm(hidden) * norm_w
    v_pool = concat([normed, memory], axis=1)            # (B, S+K, H)
    q = normed @ q_w.T ; k = v_pool @ k_w.T
    logits = q @ k.T / sqrt(H)  with causal mask on the local S block,
    memory columns fully visible
    attn = softmax(logits); h~ = attn @ v_pool
    alpha = sigmoid(hidden @ gate_w.T + gate_b)
    out = (1-alpha)*hidden + alpha*h~

Key algebraic restructure: k is never materialized.
    scores = q @ k.T = q @ k_w @ v_pool.T = q'' @ [nrm | mem/w].T
with q'' = q @ (k_w * norm_w[None,:]) and nrm = x * rstd (no norm_w);
the norm_w factor is folded out of v as well:  h~ = (attn @ [nrm|mem/w]) * w.
This deletes the (S+K) x H x H k-projection per core; only own-query
projections remain (q, q'', gate), all computed in bf16 at full PE rate.

Sharding: 8 cores = (batch b, parity h).  Core (b,h) owns the 8 query
stripes s = h, h+2, ..., h+14 (128 rows each) — interleaving balances the
causal triangle exactly.  The host stages x rows owned-stripes-first, so
the SPMD program is uniform: for slot k the score loop visits positions
0..k (own parity, diag tri mask at p==k) and 8..8+k (other parity, where
position 8+k is fully-masked on h=0 / fully-visible on h=1 via a per-core
bias column), plus the 64 memory rows.

All attention operands live in SBUF in bf16 (v, normedT, q''T, exp tiles);
the only HBM traffic is x (streamed twice), the three H^2 weights (bf16),
and the output.
"""
import numpy as np

import concourse.bass as bass
import concourse.mybir as mybir
import concourse.tile as tile
from concourse.bass_utils import run_bass_kernel_spmd

F32 = mybir.dt.float32
BF16 = mybir.dt.bfloat16
AFT = mybir.ActivationFunctionType

P = 128
H = 1024
NJ = H // P           # h blocks
NS = 8                # owned query stripes (slots) per core
NPOS = 16             # sequence stripes per batch
T_MEM = 64
SCALE = 1.0 / 32.0    # 1/sqrt(H)
EPS = 1e-6
NEG = -1.0e30

N_CORES = 8
B_FULL, S_FULL = 4, 2048


# ---------------------------------------------------------------- walrus fix
ENGINE_ATTR = {
    mybir.EngineType.PE: "tensor",
    mybir.EngineType.Activation: "scalar",
    mybir.EngineType.DVE: "vector",
    mybir.EngineType.Pool: "gpsimd",
    mybir.EngineType.SP: "sync",
}
DMA_OPS = ("InstDMACopy", "InstDMATranspose", "InstTensorLoad", "InstTensorSave",
           "InstCollectiveCompute")


def split_multi_waits(nc, limit=1, dma_limit=None):
    """This walrus build rejects engine instructions carrying more than one
    sem wait; hoist extras onto same-engine NOPs inserted just before."""
    n_split = 0
    for f in nc.m.functions:
        for blk in f.blocks:
            il = blk.instructions
            i = 0
            while i < len(il):
                ins = il[i]
                is_dma = type(ins).__name__ in DMA_OPS
                lim = dma_limit if is_dma else limit
                si = ins.sync_info
                waits = list(si.on_wait) if si is not None and si.on_wait else []
                if lim is not None and len(waits) > lim:
                    keep, extra = waits[:lim], waits[lim:]
                    si.on_wait.clear()
                    for w in keep:
                        si.on_wait.append(w)
                    eng = getattr(nc, ENGINE_ATTR[ins.engine])
                    for w in extra:
                        nop = eng.nop(nofuse=True, hint="wait_split")
                        nop.wait_op(bass.SemaphoreHandle(w.ant_name, w.id),
                                    w.wait_value, "sem-ge")
                        popped = nc.cur_bb.bb.instructions.pop()
                        assert popped.name == nop.ins.name
                        il.insert(i, nop.ins)
                        i += 1
                        n_split += 1
                i += 1
    return n_split


# ---------------------------------------------------------------- program
def build_nc():
    nc = bass.Bass()
    x_perm = nc.declare_dram_parameter("x_perm", [NPOS * P, H], F32,
                                       isOutput=False)
    dpb = lambda n, shp: nc.declare_dram_parameter(n, shp, BF16, isOutput=False)
    w2 = dpb("w2", [H, H])         # (q_w*norm_w).T @ (k_w*norm_w)
    gwT = dpb("gwT", [H, H])       # gate_w.T
    memT2 = dpb("memT2", [H, T_MEM])   # (mem / norm_w).T
    mem2 = dpb("mem2", [T_MEM, H])     # mem / norm_w
    ident = dpb("ident", [P, P])
    dpf = lambda n, shp: nc.declare_dram_parameter(n, shp, F32, isOutput=False)
    w_bc = dpf("w_bc", [P, H])     # norm_w broadcast
    b_bc = dpf("b_bc", [P, H])     # gate_b broadcast
    pbias = dpf("pbias", [P, 1])   # -1e30 (h=0) or 0 (h=1)
    tri = dpf("tri", [P, P])       # additive mask: 0 if col>=row else -1e30
    out = nc.declare_dram_parameter("out", [NS * P, H], F32, isOutput=True)

    with tile.TileContext(nc) as tc:
        from contextlib import ExitStack
        with ExitStack() as ctx:
            # ---- long-lived pools
            const = ctx.enter_context(tc.tile_pool(name="const", bufs=1))
            eps_t = const.tile([P, 1], F32)
            nc.vector.memset(eps_t[:], EPS)
            ones_b = const.tile([P, 1], BF16)
            nc.vector.memset(ones_b[:], 1.0)
            pb_t = const.tile([P, 1], F32)
            nc.sync.dma_start(out=pb_t[:], in_=pbias[:])
            tri_t = const.tile([P, P], F32)
            nc.sync.dma_start(out=tri_t[:], in_=tri[:])
            id_t = const.tile([P, P], BF16)
            nc.sync.dma_start(out=id_t[:], in_=ident[:])
            w_bc_t = const.tile([P, H], F32)
            nc.sync.dma_start(out=w_bc_t[:], in_=w_bc[:])
            std_all = const.tile([P, NPOS], F32)
            rstd_all = const.tile([P, NPOS], F32)
            rden = const.tile([P, NS], F32)
            memT_t = [const.tile([P, T_MEM], BF16, tag=f"mT{m}", name=f"mT{m}")
                      for m in range(NJ)]
            for m in range(NJ):
                nc.sync.dma_start(out=memT_t[m][:],
                                  in_=memT2[m * P:(m + 1) * P, :])

            vres = ctx.enter_context(tc.tile_pool(name="vres", bufs=1))
            v_nat = [vres.tile([P, H], BF16, tag=f"v{i}", name=f"v{i}")
                     for i in range(NPOS)]
            vmem = vres.tile([T_MEM, H], BF16)
            nc.sync.dma_start(out=vmem[:], in_=mem2[:])

            proj = ctx.enter_context(tc.tile_pool(name="proj", bufs=1))
            qsT = [proj.tile([P, NS * P], BF16, tag=f"qsT{m}", name=f"qsT{m}")
                   for m in range(NJ)]
            alpha = [proj.tile([P, H], F32, tag=f"al{i}", name=f"al{i}")
                     for i in range(NS)]

            etp = ctx.enter_context(tc.tile_pool(name="etp", bufs=1))
            et_e = [etp.tile([P, (NS - p) * P], BF16, tag=f"ete{p}",
                             name=f"ete{p}") for p in range(NS)]
            et_o = [etp.tile([P, (NS - p) * P], BF16, tag=f"eto{p}",
                             name=f"eto{p}") for p in range(NS)]
            et_m = etp.tile([T_MEM, NS * P], BF16)

            # ================= phase A + B1 under normedT lifetime
            with tc.tile_pool(name="ntp", bufs=1) as ntp:
                normedT = [ntp.tile([P, NPOS * P], BF16, tag=f"nT{j}",
                                    name=f"nT{j}") for j in range(NJ)]

                # ---- A1: stats, nrm (=v), PE transpose into normedT
                with tc.tile_pool(name="a1s", bufs=3) as ast, \
                     tc.tile_pool(name="a1ps", bufs=4, space="PSUM") as aps:
                    sq = ast.tile([P, H], F32, bufs=1)  # Square scratch
                    for idx in range(NPOS):
                        xt = ast.tile([P, H], F32, tag="xt")
                        nc.sync.dma_start(out=xt[:],
                                          in_=x_perm[idx * P:(idx + 1) * P, :])
                        ss = ast.tile([P, 1], F32, tag="ss")
                        nc.scalar.activation(sq[:], xt[:], AFT.Square,
                                             accum_out=ss[:])
                        nc.scalar.activation(std_all[:, idx:idx + 1], ss[:],
                                             AFT.Sqrt, scale=1.0 / H,
                                             bias=eps_t[:])
                        nc.vector.reciprocal(rstd_all[:, idx:idx + 1],
                                             std_all[:, idx:idx + 1])
                        nc.scalar.activation(v_nat[idx][:], xt[:], AFT.Copy,
                                             scale=rstd_all[:, idx:idx + 1])
                        for j in range(NJ):
                            tp = aps.tile([P, P], BF16, tag="tp")
                            nc.tensor.transpose(
                                tp[:], v_nat[idx][:, j * P:(j + 1) * P],
                                id_t[:])
                            nc.vector.tensor_copy(
                                normedT[j][:, idx * P:(idx + 1) * P], tp[:])

                # ---- A2: q''T directly via host-precomputed W2
                with tc.tile_pool(name="a2w", bufs=1) as wsp, \
                     tc.tile_pool(name="a2ps", bufs=2, space="PSUM") as ps2:
                    w2_s = [wsp.tile([P, H], BF16, tag=f"w2{j}",
                                     name=f"w2{j}") for j in range(NJ)]
                    for j in range(NJ):
                        nc.sync.dma_start(out=w2_s[j][:],
                                          in_=w2[j * P:(j + 1) * P, :])
                    for m in range(NJ):
                        pq = [ps2.tile([P, 512], F32, tag=f"pq{sc}",
                                       name=f"pq{sc}") for sc in range(2)]
                        for j in range(NJ):
                            for sc in range(2):
                                nc.tensor.matmul(
                                    pq[sc][:],
                                    w2_s[j][:, m * P:(m + 1) * P],
                                    normedT[j][:, sc * 512:(sc + 1) * 512],
                                    start=(j == 0), stop=(j == NJ - 1))
                        for sc in range(2):
                            nc.vector.tensor_copy(
                                qsT[m][:, sc * 512:(sc + 1) * 512],
                                pq[sc][:])

                # ---- A4: gate -> alpha (gate = (nrm @ gwT) * std)
                with tc.tile_pool(name="a4w", bufs=1) as wsp, \
                     tc.tile_pool(name="a4s", bufs=2) as gst, \
                     tc.tile_pool(name="a4ps", bufs=2, space="PSUM") as gps:
                    gw_s = [wsp.tile([P, H], BF16, tag=f"gw{j}", name=f"gw{j}")
                            for j in range(NJ)]
                    for j in range(NJ):
                        nc.sync.dma_start(out=gw_s[j][:],
                                          in_=gwT[j * P:(j + 1) * P, :])
                    b_bc_t = wsp.tile([P, H], F32)
                    nc.sync.dma_start(out=b_bc_t[:], in_=b_bc[:])
                    for si in range(NS):
                        pg = [gps.tile([P, 512], F32, tag=f"pg{oc}",
                                       name=f"pg{oc}") for oc in range(2)]
                        for j in range(NJ):
                            for oc in range(2):
                                nc.tensor.matmul(
                                    pg[oc][:],
                                    normedT[j][:, si * P:(si + 1) * P],
                                    gw_s[j][:, oc * 512:(oc + 1) * 512],
                                    start=(j == 0), stop=(j == NJ - 1))
                        for oc in range(2):
                            gl = gst.tile([P, 512], F32, tag="gl")
                            nc.scalar.activation(gl[:], pg[oc][:], AFT.Copy,
                                                 scale=std_all[:, si:si + 1])
                            nc.vector.tensor_add(
                                gl[:], gl[:],
                                b_bc_t[:, oc * 512:(oc + 1) * 512])
                            nc.scalar.activation(
                                alpha[si][:, oc * 512:(oc + 1) * 512], gl[:],
                                AFT.Sigmoid)

                # ---- B1: scores^T -> exp tiles (SBUF, bf16)
                with tc.tile_pool(name="b1ps", bufs=4, space="PSUM") as bps:
                    # memory rows first (unblocks every slot in B2)
                    for c in range(2):
                        pm = bps.tile([P, 512], F32, tag="ps")
                        for m in range(NJ):
                            nc.tensor.matmul(
                                pm[:T_MEM, :], memT_t[m][:],
                                qsT[m][:, c * 512:(c + 1) * 512],
                                start=(m == 0), stop=(m == NJ - 1))
                        nc.scalar.activation(
                            et_m[:, c * 512:(c + 1) * 512], pm[:T_MEM, :],
                            AFT.Exp, scale=SCALE)
                    # interleave parities so B2 slot k unblocks early
                    for p in range(NS):
                        w = (NS - p) * P
                        for half in range(2):   # 0 = own parity, 1 = other
                            pos = p + half * NS
                            nch = (w + 511) // 512
                            pc = []
                            for c in range(nch):
                                c0, c1 = c * 512, min(w, (c + 1) * 512)
                                psx = bps.tile([P, 512], F32, tag="ps",
                                               name="psx")
                                pc.append((psx, c0, c1))
                            for m in range(NJ):
                                for (psx, c0, c1) in pc:
                                    nc.tensor.matmul(
                                        psx[:, :c1 - c0],
                                        normedT[m][:, pos * P:(pos + 1) * P],
                                        qsT[m][:, p * P + c0:p * P + c1],
                                        start=(m == 0), stop=(m == NJ - 1),
                                        skip_group_check=True)
                            et = et_e[p] if half == 0 else et_o[p]
                            for c, (psx, c0, c1) in enumerate(pc):
                                if c == 0:
                                    if half == 0:
                                        # diagonal block: causal tri mask
                                        nc.vector.tensor_add(
                                            psx[:, 0:P], psx[:, 0:P], tri_t[:])
                                        nc.scalar.activation(
                                            et[:, c0:c1], psx[:, :c1 - c0],
                                            AFT.Exp, scale=SCALE)
                                    else:
                                        # other-parity same-index stripe:
                                        # fully masked (h=0) / visible (h=1)
                                        nc.scalar.activation(
                                            et[:, 0:P], psx[:, 0:P],
                                            AFT.Exp, scale=SCALE,
                                            bias=pb_t[:])
                                        if c1 > P:
                                            nc.scalar.activation(
                                                et[:, P:c1], psx[:, P:c1 - c0],
                                                AFT.Exp, scale=SCALE)
                                else:
                                    nc.scalar.activation(
                                        et[:, c0:c1], psx[:, :c1 - c0],
                                        AFT.Exp, scale=SCALE)

            # ================= B2: h~ accumulation + combine
            with tc.tile_pool(name="b2s", bufs=2) as bst, \
                 tc.tile_pool(name="b2ps", bufs=2, space="PSUM") as bph:
                for k in range(NS):
                    ph = [bph.tile([P, 512], F32, tag=f"ph{hc}",
                                   name=f"ph{hc}") for hc in range(2)]
                    pd = bph.tile([P, 1], F32, tag="pd")
                    stat = []
                    for p in range(k + 1):
                        stat.append((et_e[p], (k - p) * P, v_nat[p], P))
                        stat.append((et_o[p], (k - p) * P, v_nat[NS + p], P))
                    stat.append((et_m, k * P, vmem, T_MEM))
                    for ti, (et_t, c0, vt, rows) in enumerate(stat):
                        first, last = ti == 0, ti == len(stat) - 1
                        for hc in range(2):
                            nc.tensor.matmul(
                                ph[hc][:], et_t[:rows, c0:c0 + P],
                                vt[:rows, hc * 512:(hc + 1) * 512],
                                start=first, stop=last,
                                skip_group_check=True)
                        nc.tensor.matmul(
                            pd[:], et_t[:rows, c0:c0 + P], ones_b[:rows, :],
                            start=first, stop=last, skip_group_check=True)
                    nc.vector.reciprocal(rden[:, k:k + 1], pd[:])
                    hsb = bst.tile([P, H], F32, tag="hsb")
                    for hc in range(2):
                        nc.scalar.activation(
                            hsb[:, hc * 512:(hc + 1) * 512], ph[hc][:],
                            AFT.Copy, scale=rden[:, k:k + 1])
                    xs = bst.tile([P, H], F32, tag="xs")
                    nc.sync.dma_start(out=xs[:],
                                      in_=x_perm[k * P:(k + 1) * P, :])
                    nc.vector.tensor_mul(hsb[:], hsb[:], w_bc_t[:])
                    nc.vector.tensor_sub(hsb[:], hsb[:], xs[:])
                    nc.vector.tensor_mul(hsb[:], hsb[:], alpha[k][:])
                    nc.vector.tensor_add(hsb[:], hsb[:], xs[:])
                    nc.sync.dma_start(out=out[k * P:(k + 1) * P, :],
                                      in_=hsb[:])

    import os
    if os.environ.get("NO_WAIT_SPLIT") != "1":
        split_multi_waits(nc, limit=1, dma_limit=1)
    return nc


_NC_CACHE = None
_LAST_IN_MAPS = None


def _get_nc():
    global _NC_CACHE
    if _NC_CACHE is None:
        _NC_CACHE = build_nc()
    return _NC_CACHE


def prepare_in_maps(hidden_states, memory_state, q_w, k_w, norm_w, gate_w,
                    gate_b):
    import ml_dtypes
    bf = ml_dtypes.bfloat16
    hidden_states = np.asarray(hidden_states, dtype=np.float32)
    memory_state = np.asarray(memory_state, dtype=np.float32)
    q_w = np.asarray(q_w, dtype=np.float32)
    k_w = np.asarray(k_w, dtype=np.float32)
    norm_w = np.asarray(norm_w, dtype=np.float32)
    gate_w = np.asarray(gate_w, dtype=np.float32)
    gate_b = np.asarray(gate_b, dtype=np.float32)

    qwT = (q_w * norm_w[None, :]).T
    kw2 = k_w * norm_w[None, :]
    w2 = np.ascontiguousarray(qwT @ kw2).astype(bf)
    gwT = np.ascontiguousarray(gate_w.T).astype(bf)
    w_bc = np.ascontiguousarray(np.broadcast_to(norm_w, (P, H)))
    b_bc = np.ascontiguousarray(np.broadcast_to(gate_b, (P, H)))
    tri = np.where(np.arange(P)[None, :] >= np.arange(P)[:, None],
                   np.float32(0.0), np.float32(NEG)).astype(np.float32)
    ident = np.eye(P, dtype=np.float32).astype(bf)
    wsafe = np.where(np.abs(norm_w) > 1e-8, norm_w, 1.0)

    xr = hidden_states.reshape(B_FULL, NPOS, P, H)
    in_maps = []
    for c in range(N_CORES):
        b, h = divmod(c, 2)
        perm = list(range(h, NPOS, 2)) + list(range(1 - h, NPOS, 2))
        x_perm = np.ascontiguousarray(xr[b][perm].reshape(NPOS * P, H))
        mem2 = np.ascontiguousarray(memory_state[b] / wsafe[None, :]).astype(bf)
        in_maps.append({
            "x_perm": x_perm,
            "w2": w2, "gwT": gwT,
            "memT2": np.ascontiguousarray(mem2.T),
            "mem2": mem2,
            "w_bc": w_bc, "b_bc": b_bc,
            "pbias": np.full((P, 1), NEG if h == 0 else 0.0, np.float32),
            "tri": tri, "ident": ident,
        })
    return in_maps


def kernel(**inputs):
    in_maps = prepare_in_maps(**inputs)
    global _LAST_IN_MAPS
    _LAST_IN_MAPS = in_maps
    nc = _get_nc()
    res = run_bass_kernel_spmd(nc, in_maps, list(range(N_CORES)))
    out = np.empty((B_FULL, S_FULL, H), dtype=np.float32)
    for c in range(N_CORES):
        b, h = divmod(c, 2)
        o = res.results[c]["out"].reshape(NS, P, H)
        for k in range(NS):
            s = 2 * k + h
            out[b, s * P:(s + 1) * P] = o[k]
    return out
